# revision 35
# baseline (speedup 1.0000x reference)
"""Trainium2 Bass kernel for nn_MidAttnBlock (res-block -> full LxL attention -> res-block).

Contract: kernel(**inputs) takes the FULL inputs of reference.setup_inputs()
(x: (16,256,2048) f32, t: (16,256,1) f32, plus conv/groupnorm/linear params)
and returns the FULL (16,256,2048) f32 output.  Data-parallel over batch on
8 NeuronCores, 2 samples per core; each core runs an identical Bass program.

v3: conv path in bf16 (full-rate PE), attention score/denominator/attn*V
matmuls in fp8e4 DoubleRow (K=256 per instruction, ~215ns for N=512).
exp(s/16 - 4) keeps fp8e4 (max 240) unsaturated; the shift cancels in
softmax.  The two samples' phases are emitted interleaved and each phase
group owns its own PSUM banks so the engine FIFOs never serialize one
sample behind the other:
  scores A [128,1024] + scores B [128,512] (3 banks, alternating rounds),
  psav [128,512] (1), per-sample accum bank (2), kqv eviction pipe
  2x[128,1024] (2) = 8 banks.

Self-contained: all shapes/sharding hardcoded.
"""

import json as _json

import numpy as np

import concourse.bass as bass
import concourse.bass2jax as _b2j
import concourse.bass_utils as _bu
import concourse.tile as tile
from concourse import mybir
from concourse.vector_clock import ScopedClock, VectorClock


def _split_bir_waits(bir_json):
    """The walrus_driver in this container encodes at most ONE sync-wait per
    instruction (and none on Drain).  Tile's sem assigner attaches several.
    Rewrite the BIR: excess waits move to single-wait NoOps inserted directly
    before the instruction on the same engine."""
    m = _json.loads(bir_json)
    ctr = 0
    for fn in m.get("functions", []):
        for bb in fn.get("blocks", []):
            out = []
            for ins in bb.get("instructions", []):
                si = ins.get("sync_info")
                waits = (si or {}).get("on_wait") or []
                keep = 0 if ins.get("opcode") == "Drain" else 1
                if len(waits) > keep:
                    nmove = len(waits) - keep
                    for w in waits[:nmove]:
                        ctr += 1
                        out.append({
                            "debug": ins.get("debug", 0),
                            "engine": ins["engine"],
                            "ins": [],
                            "name": f"{ins['name']}-wsp{ctr}",
                            "opcode": "NoOp",
                            "outs": [],
                            "sync_info": {"on_update": [], "on_wait": [w]},
                        })
                    si["on_wait"] = waits[nmove:]
                out.append(ins)
            bb["instructions"] = out
    return _json.dumps(m).encode()


_orig_compile_bir_kernel = _bu.compile_bir_kernel


def _compile_bir_splitwaits(bir_json, tmpdir, neff_name="file.neff"):
    return _orig_compile_bir_kernel(_split_bir_waits(bir_json), tmpdir, neff_name)


if getattr(_bu.compile_bir_kernel, "__name__", "") != "_compile_bir_splitwaits":
    _bu.compile_bir_kernel = _compile_bir_splitwaits
    _b2j.compile_bir_kernel = _compile_bir_splitwaits


F32 = mybir.dt.float32
F32R = mybir.dt.float32r
BF16 = mybir.dt.bfloat16
F8 = mybir.dt.float8e4
AF = mybir.ActivationFunctionType
OP = mybir.AluOpType
DR = mybir.MatmulPerfMode.DoubleRow

P = 128          # partitions
C = 256          # channels
CB = 2           # channel blocks of 128
L = 2048         # sequence length
LS = 512         # l-slice (matmul moving dim)
NL = L // LS     # 4 slices
KB = L // P      # 16 k-blocks for attention
GPB = 16         # groups per channel-block (32 groups, 8 ch each)
EPS = 1e-5
S = 2            # samples per core
NCORES = 8
SCALE = 1.0 / 16.0   # 1/sqrt(C)
EXPSHIFT = -4.0      # exp(s/16 - 4): keeps fp8e4 (max 240) unsaturated
NWARM = 90           # PE warm-up matmuls at t=0 (HAM un-throttle)

# attention score rounds: (first_kb, n_kb) alternating between the 2-kb pool A
# and the 1-kb pool B so exp can pipeline against the score matmuls.
_ROUNDS = [(0, 2), (2, 2), (4, 2), (6, 2), (8, 2), (10, 2), (12, 2), (14, 2)]
assert sum(n for _, n in _ROUNDS) == KB


class _TileContextPatched(tile.TileContext):
    """TileContext whose kernel-tail drain carries no sem waits (the container
    walrus rejects waits on Drain); one SP NOP per proc carries them instead."""

    def _drain_and_barrier(self, tick_clock, wait_clock):
        gc = tick_clock.global_clock
        n = len(gc)
        for p in range(n):
            v = gc[p]
            if v > 0:
                vec = [0] * n
                vec[p] = v
                nop = self.nc.sync.nop()
                wait_clock.add_sem_waits(nop.ins, ScopedClock({None: VectorClock(vec)}))
        self.nc.sync.drain()
        self.nc.all_engine_barrier()
        assert self.sems is not None
        popped = self.nc._tile_sem_poison_stack.pop()
        assert popped is self._sem_poison
        self.nc.clear_and_free_semaphores(list(self.sems.allocated().values()))
        self.nc.all_engine_barrier()


def build_program(samples=S, use_bias=()):
    """Build the per-core Bass program (identical on all cores)."""
    assert "linb" not in use_bias, "nonzero lin_b not supported"
    nc = bass.Bass()

    # ---- DRAM I/O (per core) ----
    x_d = nc.dram_tensor("x", (samples, C, L), F32, kind="ExternalInput")
    t_d = nc.dram_tensor("tv", (samples, P, CB, 2), F32, kind="ExternalInput")
    w_conv = {}
    for rb in ("r1", "r2"):
        w_conv[rb, 1] = nc.dram_tensor(f"{rb}_w1t", (P, CB, 3, C), BF16, kind="ExternalInput")
        w_conv[rb, 2] = nc.dram_tensor(f"{rb}_w2t", (P, CB, 3, C), BF16, kind="ExternalInput")
    wkqv_d = nc.dram_tensor("wkqvt", (P, CB, 3 * C), BF16, kind="ExternalInput")
    gnw_d = {}
    for rb in ("r1", "r2"):
        for ln in (1, 2):
            gnw_d[rb, ln, "w"] = nc.dram_tensor(f"{rb}_gn{ln}_ws", (P, CB), F32, kind="ExternalInput")
            gnw_d[rb, ln, "b"] = nc.dram_tensor(f"{rb}_gn{ln}_bs", (P, CB), F32, kind="ExternalInput")
    c2b_d = {}
    if "c2b_r1" in use_bias:
        c2b_d["r1"] = nc.dram_tensor("r1_c2bs", (P, CB), F32, kind="ExternalInput")
    if "c2b_r2" in use_bias:
        c2b_d["r2"] = nc.dram_tensor("r2_c2bs", (P, CB), F32, kind="ExternalInput")
    gind_d = nc.dram_tensor("gind", (P, GPB), F32R, kind="ExternalInput")
    bind_d = nc.dram_tensor("bind", (CB, P, P), F32R, kind="ExternalInput")
    po_d = nc.dram_tensor("po", (P, 2, 16), F8, kind="ExternalInput")
    onesr_d = nc.dram_tensor("onesr", (1, P), F32R, kind="ExternalInput")
    out_d = nc.dram_tensor("out", (samples, C, L), F32, kind="ExternalOutput")
    warm_d = nc.dram_tensor("warm", (P, 4), F32, kind="ExternalOutput")

    from contextlib import ExitStack
    with ExitStack() as _stk:
        tc = _stk.enter_context(_TileContextPatched(nc))
        _pool = lambda **kw: _stk.enter_context(tc.tile_pool(**kw))
        consts = _pool(name="consts", bufs=1)
        xpp = _pool(name="xpp", bufs=1)
        actp = _pool(name="actp", bufs=4)
        hp = _pool(name="hp", bufs=2)
        avp = _pool(name="avp", bufs=1)
        x1p = _pool(name="x1p", bufs=1)
        kqvp = _pool(name="kqvp", bufs=1)
        expp = _pool(name="expp", bufs=3)
        outp = _pool(name="outp", bufs=4)
        rdbp = _pool(name="rdbp", bufs=2)
        small = _pool(name="small", bufs=4)
        t2p = _pool(name="t2p", bufs=1)
        spsa = _pool(name="spsa", bufs=2, space="PSUM")
        macc = _pool(name="macc", bufs=2, space="PSUM")
        pana = _pool(name="pana", bufs=1, space="PSUM")
        panb = _pool(name="panb", bufs=1, space="PSUM")

        pan = [pana, panb]  # per-sample accumulation bank

        # ---- warm-up: keep PE busy (and HAM un-throttled) during input DMA ----
        wsrc = consts.tile([P, LS], BF16, tag="wsrc", name="wsrc")
        nc.vector.memset(wsrc[:], 0.0)
        wps = macc.tile([P, LS], F32, tag="m", name="wps")
        for i in range(NWARM):
            nc.tensor.matmul(wps[:], wsrc[:, :P], wsrc[:], start=(i == 0), stop=(i == NWARM - 1))
        wsb = consts.tile([P, 4], F32, tag="wsb", name="wsb")
        nc.vector.tensor_copy(out=wsb[:], in_=wps[:, :4])
        nc.gpsimd.dma_start(warm_d[:], wsb[:])

        # ---- input x: spread DMA across the 3 DMA-capable queues, s0 first ----
        dmaq = [nc.sync, nc.scalar, nc.gpsimd]
        xp = {}
        for s in range(samples):
            for cb in range(CB):
                xp[s, cb] = xpp.tile([P, L], F32, tag=f"xp{s}{cb}", name=f"xp{s}{cb}")
        t2 = {}
        for s in range(samples):
            t2[s] = t2p.tile([P, CB, 2], F32, tag=f"t2{s}", name=f"t2{s}")
            nc.sync.dma_start(t2[s][:], t_d[s])
            qi = 0
            _QPAT = (1, 2, 0, 1, 2, 1, 2, 0)  # scalar/gpsimd HW queues take more
            for cb in range(CB):
                for i in range(2):
                    dmaq[_QPAT[qi % 8]].dma_start(
                        xp[s, cb][:, i * 2 * LS : (i + 1) * 2 * LS],
                        x_d[s, cb * P : (cb + 1) * P, i * 2 * LS : (i + 1) * 2 * LS],
                    )
                    qi += 1

        # ---- persistent constants / weights in SBUF (after x in queue order) ----
        w1_sb = {}
        w2_sb = {}
        for rb in ("r1", "r2"):
            w1_sb[rb] = consts.tile([P, CB, 3, C], BF16, tag=f"w1_{rb}", name=f"w1_{rb}")
            nc.scalar.dma_start(w1_sb[rb][:], w_conv[rb, 1][:])
            w2_sb[rb] = consts.tile([P, CB, 3, C], BF16, tag=f"w2_{rb}", name=f"w2_{rb}")
            nc.gpsimd.dma_start(w2_sb[rb][:], w_conv[rb, 2][:])
        wkqv_sb = consts.tile([P, CB, 3 * C], BF16, tag="wkqv", name="wkqv")
        nc.scalar.dma_start(wkqv_sb[:], wkqv_d[:])
        gnp_sb = {}
        for rb in ("r1", "r2"):
            for ln in (1, 2):
                for wb in ("w", "b"):
                    tl = consts.tile([P, CB], F32, tag=f"gn_{rb}{ln}{wb}", name=f"gn_{rb}{ln}{wb}")
                    nc.gpsimd.dma_start(tl[:], gnw_d[rb, ln, wb][:])
                    gnp_sb[rb, ln, wb] = tl
        c2b_sb = {}
        for rb, dten in c2b_d.items():
            c2b_sb[rb] = consts.tile([P, CB], F32, tag=f"c2b_{rb}", name=f"c2b_{rb}")
            nc.gpsimd.dma_start(c2b_sb[rb][:], dten[:])
        gind_sb = consts.tile([P, GPB], F32R, tag="gind", name="gind")
        nc.gpsimd.dma_start(gind_sb[:], gind_d[:])
        bind_sb = consts.tile([P, CB, P], F32R, tag="bind", name="bind")
        nc.gpsimd.dma_start(bind_sb[:], bind_d.rearrange("cb p c -> p cb c"))
        po_sb = consts.tile([P, 2, 16], F8, tag="po", name="po")
        nc.gpsimd.dma_start(po_sb[:], po_d[:])
        onesr_sb = consts.tile([1, P], F32R, tag="onesr", name="onesr")
        nc.gpsimd.dma_start(onesr_sb[:], onesr_d[:])
        eps_sb = consts.tile([P, 1], F32, tag="eps", name="eps")
        nc.vector.memset(eps_sb[:], EPS)
        expb_sb = consts.tile([P, 1], F32, tag="expb", name="expb")
        nc.vector.memset(expb_sb[:], EXPSHIFT)
        zero2 = consts.tile([P, 2], F32, tag="zero2", name="zero2")
        nc.vector.memset(zero2[:], 0.0)

        def alloc_act(tag):
            ts = []
            for cb in range(CB):
                tl = actp.tile([P, L + 2], BF16, tag=f"{tag}{cb}", name=f"{tag}{cb}")
                nc.vector.memset(tl[:, 0:1], 0.0)
                nc.vector.memset(tl[:, L + 1 : L + 2], 0.0)
                ts.append(tl)
            return ts

        def gn_relu(s, src, dst, rb, ln, dve_apply=False):
            """dst (padded bf16 act pair) = relu(groupnorm(src) * w + b)."""
            gp = []
            for cb in range(CB):
                stats = small.tile([P, NL, 6], F32, tag="stats", name="stats")
                for i in range(NL):
                    nc.vector.bn_stats(out=stats[:, i, :], in_=src[cb][:, i * LS : (i + 1) * LS])
                mv = small.tile([P, 2], F32, tag="mv", name="mv")
                nc.vector.bn_aggr(out=mv[:], in_=stats[:])
                tmp = small.tile([P, 2], F32R, tag="tmp", name="tmp")
                nc.vector.tensor_copy(out=tmp[:, 0:1], in_=mv[:, 0:1])
                nc.vector.tensor_tensor(out=tmp[:, 1:2], in0=mv[:, 0:1], in1=mv[:, 0:1], op=OP.mult)
                nc.vector.tensor_tensor(out=tmp[:, 1:2], in0=tmp[:, 1:2].bitcast(F32), in1=mv[:, 1:2], op=OP.add)
                g = pan[s].tile([GPB, 2], F32, tag="a", name="gp")
                nc.tensor.matmul(g[:], gind_sb[:], tmp[:], start=True, stop=True)
                gp.append(g)
            NG = 32 * CB
            gs = small.tile([NG, 2], F32, tag="gs", name="gs")
            nc.vector.tensor_copy(out=gs[:], in_=zero2[:NG])
            for cb in range(CB):
                nc.vector.tensor_copy(out=gs[cb * 32 : cb * 32 + GPB, :], in_=gp[cb][:])
            var = small.tile([NG, 1], F32, tag="var", name="var")
            nc.vector.tensor_tensor(out=var[:], in0=gs[:, 0:1], in1=gs[:, 0:1], op=OP.mult)
            nc.vector.tensor_tensor(out=var[:], in0=gs[:, 1:2], in1=var[:], op=OP.subtract)
            nc.scalar.activation(out=var[:], in_=var[:], func=AF.Ln, bias=eps_sb[:NG])
            gpk = small.tile([P, 2], F32R, tag="gpk", name="gpk")
            nc.vector.tensor_copy(out=gpk[:], in_=zero2[:])
            nc.scalar.activation(out=gpk[:NG, 0:1], in_=var[:], func=AF.Exp, scale=-0.5)
            nc.vector.tensor_scalar_mul(gpk[:NG, 1:2], gs[:, 0:1], -1.0)
            for cb in range(CB):
                bc = pan[s].tile([P, 2], F32, tag="a", name="bc")
                nc.tensor.matmul(bc[:], bind_sb[:, cb, :], gpk[:], start=True, stop=True)
                sb = small.tile([P, 2], F32, tag="sb", name="sb")
                nc.vector.tensor_scalar_mul(sb[:, 0:1], bc[:, 0:1], gnp_sb[rb, ln, "w"][:, cb : cb + 1])
                nc.vector.tensor_tensor(out=sb[:, 1:2], in0=bc[:, 1:2], in1=sb[:, 0:1], op=OP.mult)
                nc.vector.tensor_scalar_add(sb[:, 1:2], sb[:, 1:2], gnp_sb[rb, ln, "b"][:, cb : cb + 1])
                for i in range(2):
                    dsl = dst[cb][:, 1 + i * 2 * LS : 1 + (i + 1) * 2 * LS]
                    ssl = src[cb][:, i * 2 * LS : (i + 1) * 2 * LS]
                    if dve_apply:
                        nc.vector.tensor_scalar(
                            dsl, ssl, sb[:, 0:1], sb[:, 1:2], op0=OP.mult, op1=OP.add
                        )
                        nc.vector.tensor_scalar_max(dsl, dsl, 0.0)
                    else:
                        nc.scalar.activation(
                            out=dsl, in_=ssl, func=AF.Relu,
                            bias=sb[:, 1:2], scale=sb[:, 0:1],
                        )

        def conv3(s, src, wt, consume):
            for ocb in range(CB):
                for ls in range(NL):
                    ps = macc.tile([P, LS], F32, tag="m", name="acc")
                    k = 0
                    for icb in range(CB):
                        for tap in range(3):
                            nc.tensor.matmul(
                                ps[:],
                                wt[:, icb, tap, ocb * P : (ocb + 1) * P],
                                src[icb][:, ls * LS + tap : ls * LS + tap + LS],
                                start=(k == 0),
                                stop=(k == 5),
                            )
                            k += 1
                    consume(ocb, ls, ps)

        # ---- res-block phase pieces (so emission can interleave samples) ----
        act1 = {}
        hbuf = {}
        act2 = {}
        x1 = {}

        def phase_gn1(s, rb, src):
            act1[s] = alloc_act("a")
            with nc.named_scope(f"s{s}_{rb}_gn1"):
                gn_relu(s, src, act1[s], rb, 1, dve_apply=False)

        def phase_conv1(s, rb, rbi, eat1_act):
            hbuf[s] = [hp.tile([P, L], BF16, tag=f"h{cb}", name=f"h{cb}") for cb in range(CB)]
            with nc.named_scope(f"s{s}_{rb}_conv1"):
                def eat1(ocb, ls, ps):
                    dst = hbuf[s][ocb][:, ls * LS : (ls + 1) * LS]
                    if eat1_act:
                        nc.scalar.activation(out=dst, in_=ps[:], func=AF.Identity,
                                             bias=t2[s][:, ocb, rbi : rbi + 1])
                    else:
                        nc.vector.tensor_scalar_add(dst, ps[:], t2[s][:, ocb, rbi : rbi + 1])
                conv3(s, act1[s], w1_sb[rb], eat1)

        def phase_gn2(s, rb):
            act2[s] = alloc_act("a")
            with nc.named_scope(f"s{s}_{rb}_gn2"):
                gn_relu(s, hbuf[s], act2[s], rb, 2, dve_apply=False)

        def phase_conv2(s, rb, src, final):
            res = None
            if not final:
                res = [x1p.tile([P, L], BF16, tag=f"x1{s}{cb}", name=f"x1{s}{cb}") for cb in range(CB)]
            with nc.named_scope(f"s{s}_{rb}_conv2"):
                oq = [0]
                def eat2(ocb, ls, ps):
                    if rb in c2b_sb:
                        nc.vector.tensor_scalar_add(ps[:], ps[:], c2b_sb[rb][:, ocb : ocb + 1])
                    resid = src[ocb][:, ls * LS : (ls + 1) * LS]
                    if final:
                        ot = outp.tile([P, LS], F32, tag="ot", name="ot")
                        nc.vector.tensor_tensor(out=ot[:], in0=ps[:], in1=resid, op=OP.add)
                        dmaq[oq[0] % 3].dma_start(
                            out_d[s, ocb * P : (ocb + 1) * P, ls * LS : (ls + 1) * LS], ot[:]
                        )
                        oq[0] += 1
                    else:
                        nc.vector.tensor_tensor(
                            out=res[ocb][:, ls * LS : (ls + 1) * LS],
                            in0=ps[:], in1=resid, op=OP.add,
                        )
                conv3(s, act2[s], w2_sb[rb], eat2)
            return res

        def kqv(s, x1s):
            """k,q into fp8 pair-plane layout; v into fp8 [l, kb, c] layout.
            2x[128,1024] psum pipe: 4 matmuls fill a tile, one DVE eviction."""
            ktp = kqvp.tile([P, CB, L], F8, tag=f"kt{s}", name=f"kt{s}")
            qtp = kqvp.tile([P, CB, L], F8, tag=f"qt{s}", name=f"qt{s}")
            vtp = kqvp.tile([P, KB, C], F8, tag=f"vt{s}", name=f"vt{s}")
            with nc.named_scope(f"s{s}_kqv"):
                for j, dst in ((0, ktp), (1, qtp)):
                    for ocb in range(CB):
                        off = j * C + ocb * P
                        for ls in range(NL):
                            ps = macc.tile([P, LS], F32, tag="m", name="ka")
                            for icb in range(CB):
                                nc.tensor.matmul(
                                    ps[:],
                                    wkqv_sb[:, icb, off : off + P],
                                    x1s[icb][:, ls * LS : (ls + 1) * LS],
                                    start=(icb == 0),
                                    stop=(icb == 1),
                                )
                            nc.vector.tensor_copy(
                                out=dst[:, ocb, ls * LS : (ls + 1) * LS], in_=ps[:]
                            )
                for lh in range(KB // 2):
                    ps = macc.tile([P, LS], F32, tag="m", name="ka")
                    for lsub in range(2):
                        lb = lh * 2 + lsub
                        for icb in range(CB):
                            nc.tensor.matmul(
                                ps[:, lsub * C : (lsub + 1) * C],
                                x1s[icb][:, lb * P : (lb + 1) * P],
                                wkqv_sb[:, icb, 2 * C : 3 * C],
                                start=(icb == 0),
                                stop=(icb == 1),
                            )
                    nc.vector.tensor_copy(out=vtp[:, lh * 2 : (lh + 1) * 2, :], in_=ps[:])
            return ktp, qtp, vtp

        def attn_qs(s, qs, ktp, qtp, vtp, av):
            with nc.named_scope(f"s{s}_attn{qs}"):
                ex = expp.tile([P, KB, LS], F8, tag="ex", name="ex")
                qsl = qtp[:, :, qs * LS : (qs + 1) * LS]
                # scores + exp, alternating 2-bank / 1-bank psum rounds
                for kb0, nkb in _ROUNDS:
                    sp = spsa.tile([P, nkb * LS], F32, tag="sp", name="sp")
                    for j in range(nkb):
                        kb = kb0 + j
                        nc.tensor.matmul(
                            sp[:, j * LS : (j + 1) * LS],
                            ktp[:, :, kb * P : (kb + 1) * P],
                            qsl,
                            start=True, stop=True, perf_mode=DR,
                        )
                    nc.scalar.activation(
                        out=ex[:, kb0 : kb0 + nkb, :], in_=sp[:],
                        func=AF.Exp, bias=expb_sb[:], scale=SCALE,
                    )
                # denominator (8 pair matmuls), then 1/dn broadcast
                dn = pan[s].tile([1, LS], F32, tag="a", name="dn")
                for g in range(KB // 2):
                    nc.tensor.matmul(
                        dn[:], po_sb[:, :, 0:1], ex[:, 2 * g : 2 * g + 2, :],
                        start=(g == 0), stop=(g == KB // 2 - 1), perf_mode=DR,
                    )
                lnd = small.tile([1, LS], F32, tag="lnd", name="lnd")
                nc.scalar.activation(out=lnd[:], in_=dn[:], func=AF.Ln)
                rd = small.tile([1, LS], F32R, tag="rd", name="rd")
                nc.scalar.activation(out=rd[:], in_=lnd[:], func=AF.Exp, scale=-1.0)
                rb_ps = pan[s].tile([P, LS], F32, tag="a", name="rb_ps")
                nc.tensor.matmul(rb_ps[:], onesr_sb[:], rd[:], start=True, stop=True)
                rdb = rdbp.tile([P, LS], BF16, tag="rdbs", name="rdb")
                nc.vector.tensor_copy(out=rdb[:], in_=rb_ps[:])
                # attn @ V per channel block (per-sample bank, sequential cb)
                for cb in range(CB):
                    psv = pan[s].tile([P, LS], F32, tag="a", name="psv")
                    for g in range(KB // 2):
                        nc.tensor.matmul(
                            psv[:],
                            vtp[:, 2 * g : 2 * g + 2, cb * P : (cb + 1) * P],
                            ex[:, 2 * g : 2 * g + 2, :],
                            start=(g == 0), stop=(g == KB // 2 - 1), perf_mode=DR,
                        )
                    nc.vector.tensor_tensor(
                        out=av[cb][:, qs * LS : (qs + 1) * LS],
                        in0=psv[:], in1=rdb[:], op=OP.mult,
                    )

        # ================= program body (samples interleaved) =================
        for s in range(samples):
            phase_gn1(s, "r1", [xp[s, 0], xp[s, 1]])
        for s in range(samples):
            phase_conv1(s, "r1", 0, eat1_act=False)
        for s in range(samples):
            phase_gn2(s, "r1")
        for s in range(samples):
            x1[s] = phase_conv2(s, "r1", [xp[s, 0], xp[s, 1]], final=False)
        kqvt = {}
        for s in range(samples):
            kqvt[s] = kqv(s, x1[s])
        av = {}
        for s in range(samples):
            av[s] = [avp.tile([P, L], BF16, tag=f"av{s}{cb}", name=f"av{s}{cb}") for cb in range(CB)]
        attn_qs(0, 0, *kqvt[0], av[0])
        attn_qs(0, 1, *kqvt[0], av[0])
        attn_qs(1, 0, *kqvt[1], av[1])
        attn_qs(0, 2, *kqvt[0], av[0])
        attn_qs(1, 1, *kqvt[1], av[1])
        attn_qs(0, 3, *kqvt[0], av[0])
        attn_qs(1, 2, *kqvt[1], av[1])
        phase_gn1(0, "r2", av[0])
        attn_qs(1, 3, *kqvt[1], av[1])
        phase_conv1(0, "r2", 1, eat1_act=True)
        phase_gn1(1, "r2", av[1])
        phase_gn2(0, "r2")
        phase_conv2(0, "r2", av[0], final=True)
        phase_conv1(1, "r2", 1, eat1_act=True)
        phase_gn2(1, "r2")
        phase_conv2(1, "r2", av[1], final=True)

    nc.finalize()
    return nc


def _to_bf16(a):
    a = np.ascontiguousarray(np.asarray(a, np.float32))
    u = a.view(np.uint32)
    return (((u >> 16) + ((u >> 15) & 1)) & 0xFFFF).astype(np.uint16)


def _pack_conv_w(w):
    """(O, I, 3) f32 -> [P, icb, tap, oc] bf16 (uint16 view)."""
    w = np.asarray(w, dtype=np.float32)
    t = np.ascontiguousarray(w.transpose(1, 2, 0).reshape(CB, P, 3, w.shape[0]).transpose(1, 0, 2, 3))
    return _to_bf16(t)


def _pack_gn(v):
    return np.ascontiguousarray(np.asarray(v, dtype=np.float32).reshape(CB, P).T)


def make_in_maps(inp, use_bias):
    gind = np.zeros((P, GPB), np.float32)
    bind = np.zeros((CB, P, P), np.float32)
    for cc in range(P):
        gind[cc, cc // 8] = 0.125
        for cb in range(CB):
            bind[cb, cb * 32 + cc // 8, cc] = 1.0
    po = np.full((P, 2, 16), 0x38, np.uint8)  # fp8e4 1.0
    shared = {
        "wkqvt": _to_bf16(
            inp["lin_w"][:, :, 0].T.reshape(CB, P, 3 * C).transpose(1, 0, 2)
        ),
        "gind": gind,
        "bind": bind,
        "po": po,
        "onesr": np.ones((1, P), np.float32),
    }
    for rb in ("r1", "r2"):
        shared[f"{rb}_w1t"] = _pack_conv_w(inp[f"{rb}_c1_w"])
        shared[f"{rb}_w2t"] = _pack_conv_w(inp[f"{rb}_c2_w"])
        for ln in (1, 2):
            shared[f"{rb}_gn{ln}_ws"] = _pack_gn(inp[f"{rb}_gn{ln}_w"])
            shared[f"{rb}_gn{ln}_bs"] = _pack_gn(inp[f"{rb}_gn{ln}_b"])
    if "c2b_r1" in use_bias:
        shared["r1_c2bs"] = _pack_gn(inp["r1_c2_b"])
    if "c2b_r2" in use_bias:
        shared["r2_c2bs"] = _pack_gn(inp["r2_c2_b"])

    tfull = inp["t"][:, :, 0]
    nb = inp["x"].shape[0]
    tv = np.empty((nb, P, CB, 2), np.float32)
    for rbi, rb in enumerate(("r1", "r2")):
        v = tfull + inp[f"{rb}_c1_b"][None, :]
        tv[:, :, :, rbi] = v.reshape(nb, CB, P).transpose(0, 2, 1)

    in_maps = []
    for c in range(NCORES):
        sl = slice(S * c, S * (c + 1))
        m = dict(shared)
        m["x"] = inp["x"][sl]
        m["tv"] = np.ascontiguousarray(tv[sl])
        in_maps.append(m)
    return in_maps


_CACHE = {}


def kernel(**inputs):
    inp = {k: np.ascontiguousarray(np.asarray(v, dtype=np.float32)) for k, v in inputs.items()}

    use_bias = []
    if np.any(inp["r1_c2_b"]):
        use_bias.append("c2b_r1")
    if np.any(inp["r2_c2_b"]):
        use_bias.append("c2b_r2")
    if np.any(inp["lin_b"]):
        use_bias.append("linb")
    use_bias = tuple(use_bias)

    if ("nc", use_bias) not in _CACHE:
        _CACHE[("nc", use_bias)] = build_program(S, use_bias)
    nc = _CACHE[("nc", use_bias)]

    in_maps = make_in_maps(inp, use_bias)
    res = _bu.run_bass_kernel_spmd(nc, in_maps, core_ids=list(range(NCORES)))
    out = np.concatenate([res.results[c]["out"] for c in range(NCORES)], axis=0)
    return out.astype(np.float32)


# revision 36
# speedup vs baseline: 1.0244x; 1.0244x over previous
"""Trainium2 Bass kernel for nn_MidAttnBlock (res-block -> full LxL attention -> res-block).

Contract: kernel(**inputs) takes the FULL inputs of reference.setup_inputs()
(x: (16,256,2048) f32, t: (16,256,1) f32, plus conv/groupnorm/linear params)
and returns the FULL (16,256,2048) f32 output.  Data-parallel over batch on
8 NeuronCores, 2 samples per core; each core runs an identical Bass program.

v3: conv path in bf16 (full-rate PE), attention score/denominator/attn*V
matmuls in fp8e4 DoubleRow (K=256 per instruction, ~215ns for N=512).
exp(s/16 - 4) keeps fp8e4 (max 240) unsaturated; the shift cancels in
softmax.  The two samples' phases are emitted interleaved and each phase
group owns its own PSUM banks so the engine FIFOs never serialize one
sample behind the other:
  scores A [128,1024] + scores B [128,512] (3 banks, alternating rounds),
  psav [128,512] (1), per-sample accum bank (2), kqv eviction pipe
  2x[128,1024] (2) = 8 banks.

Self-contained: all shapes/sharding hardcoded.
"""

import json as _json

import numpy as np

import concourse.bass as bass
import concourse.bass2jax as _b2j
import concourse.bass_utils as _bu
import concourse.tile as tile
from concourse import mybir
from concourse.vector_clock import ScopedClock, VectorClock


def _split_bir_waits(bir_json):
    """The walrus_driver in this container encodes at most ONE sync-wait per
    instruction (and none on Drain).  Tile's sem assigner attaches several.
    Rewrite the BIR: excess waits move to single-wait NoOps inserted directly
    before the instruction on the same engine."""
    m = _json.loads(bir_json)
    ctr = 0
    for fn in m.get("functions", []):
        for bb in fn.get("blocks", []):
            out = []
            for ins in bb.get("instructions", []):
                si = ins.get("sync_info")
                waits = (si or {}).get("on_wait") or []
                keep = 0 if ins.get("opcode") == "Drain" else 1
                if len(waits) > keep:
                    nmove = len(waits) - keep
                    for w in waits[:nmove]:
                        ctr += 1
                        out.append({
                            "debug": ins.get("debug", 0),
                            "engine": ins["engine"],
                            "ins": [],
                            "name": f"{ins['name']}-wsp{ctr}",
                            "opcode": "NoOp",
                            "outs": [],
                            "sync_info": {"on_update": [], "on_wait": [w]},
                        })
                    si["on_wait"] = waits[nmove:]
                out.append(ins)
            bb["instructions"] = out
    return _json.dumps(m).encode()


_orig_compile_bir_kernel = _bu.compile_bir_kernel


def _compile_bir_splitwaits(bir_json, tmpdir, neff_name="file.neff"):
    return _orig_compile_bir_kernel(_split_bir_waits(bir_json), tmpdir, neff_name)


if getattr(_bu.compile_bir_kernel, "__name__", "") != "_compile_bir_splitwaits":
    _bu.compile_bir_kernel = _compile_bir_splitwaits
    _b2j.compile_bir_kernel = _compile_bir_splitwaits


F32 = mybir.dt.float32
F32R = mybir.dt.float32r
BF16 = mybir.dt.bfloat16
F8 = mybir.dt.float8e4
AF = mybir.ActivationFunctionType
OP = mybir.AluOpType
DR = mybir.MatmulPerfMode.DoubleRow

P = 128          # partitions
C = 256          # channels
CB = 2           # channel blocks of 128
L = 2048         # sequence length
LS = 512         # l-slice (matmul moving dim)
NL = L // LS     # 4 slices
KB = L // P      # 16 k-blocks for attention
GPB = 16         # groups per channel-block (32 groups, 8 ch each)
EPS = 1e-5
S = 2            # samples per core
NCORES = 8
SCALE = 1.0 / 16.0   # 1/sqrt(C)
EXPSHIFT = -4.0      # exp(s/16 - 4): keeps fp8e4 (max 240) unsaturated
NWARM = 90           # PE warm-up matmuls at t=0 (HAM un-throttle)

# attention score rounds: (first_kb, n_kb) alternating between the 2-kb pool A
# and the 1-kb pool B so exp can pipeline against the score matmuls.
_ROUNDS = [(0, 2), (2, 2), (4, 2), (6, 2), (8, 2), (10, 2), (12, 2), (14, 2)]
assert sum(n for _, n in _ROUNDS) == KB


class _TileContextPatched(tile.TileContext):
    """TileContext whose kernel-tail drain carries no sem waits (the container
    walrus rejects waits on Drain); one SP NOP per proc carries them instead."""

    def _drain_and_barrier(self, tick_clock, wait_clock):
        gc = tick_clock.global_clock
        n = len(gc)
        for p in range(n):
            v = gc[p]
            if v > 0:
                vec = [0] * n
                vec[p] = v
                nop = self.nc.sync.nop()
                wait_clock.add_sem_waits(nop.ins, ScopedClock({None: VectorClock(vec)}))
        self.nc.sync.drain()
        self.nc.all_engine_barrier()
        assert self.sems is not None
        popped = self.nc._tile_sem_poison_stack.pop()
        assert popped is self._sem_poison
        self.nc.clear_and_free_semaphores(list(self.sems.allocated().values()))
        self.nc.all_engine_barrier()


def build_program(samples=S, use_bias=()):
    """Build the per-core Bass program (identical on all cores)."""
    assert "linb" not in use_bias, "nonzero lin_b not supported"
    nc = bass.Bass()

    # ---- DRAM I/O (per core) ----
    x_d = nc.dram_tensor("x", (samples, C, L), F32, kind="ExternalInput")
    t_d = nc.dram_tensor("tv", (samples, P, CB, 2), F32, kind="ExternalInput")
    w_conv = {}
    for rb in ("r1", "r2"):
        w_conv[rb, 1] = nc.dram_tensor(f"{rb}_w1t", (P, CB, 3, C), BF16, kind="ExternalInput")
        w_conv[rb, 2] = nc.dram_tensor(f"{rb}_w2t", (P, CB, 3, C), BF16, kind="ExternalInput")
    wkqv_d = nc.dram_tensor("wkqvt", (P, CB, 3 * C), BF16, kind="ExternalInput")
    gnw_d = {}
    for rb in ("r1", "r2"):
        for ln in (1, 2):
            gnw_d[rb, ln, "w"] = nc.dram_tensor(f"{rb}_gn{ln}_ws", (P, CB), F32, kind="ExternalInput")
            gnw_d[rb, ln, "b"] = nc.dram_tensor(f"{rb}_gn{ln}_bs", (P, CB), F32, kind="ExternalInput")
    c2b_d = {}
    if "c2b_r1" in use_bias:
        c2b_d["r1"] = nc.dram_tensor("r1_c2bs", (P, CB), F32, kind="ExternalInput")
    if "c2b_r2" in use_bias:
        c2b_d["r2"] = nc.dram_tensor("r2_c2bs", (P, CB), F32, kind="ExternalInput")
    gind_d = nc.dram_tensor("gind", (P, GPB), F32R, kind="ExternalInput")
    bind_d = nc.dram_tensor("bind", (CB, P, P), F32R, kind="ExternalInput")
    po_d = nc.dram_tensor("po", (P, 2, 16), F8, kind="ExternalInput")
    onesr_d = nc.dram_tensor("onesr", (1, P), F32R, kind="ExternalInput")
    out_d = nc.dram_tensor("out", (samples, C, L), F32, kind="ExternalOutput")
    warm_d = nc.dram_tensor("warm", (P, 4), F32, kind="ExternalOutput")

    from contextlib import ExitStack
    with ExitStack() as _stk:
        tc = _stk.enter_context(_TileContextPatched(nc))
        _pool = lambda **kw: _stk.enter_context(tc.tile_pool(**kw))
        consts = _pool(name="consts", bufs=1)
        xpp = _pool(name="xpp", bufs=1)
        actp = _pool(name="actp", bufs=4)
        hp = _pool(name="hp", bufs=2)
        avp = _pool(name="avp", bufs=1)
        x1p = _pool(name="x1p", bufs=1)
        kqvp = _pool(name="kqvp", bufs=1)
        expp = _pool(name="expp", bufs=3)
        outp = _pool(name="outp", bufs=4)
        rdbp = _pool(name="rdbp", bufs=2)
        small = _pool(name="small", bufs=4)
        t2p = _pool(name="t2p", bufs=1)
        spsa = _pool(name="spsa", bufs=2, space="PSUM")
        macc = _pool(name="macc", bufs=2, space="PSUM")
        pana = _pool(name="pana", bufs=1, space="PSUM")
        panb = _pool(name="panb", bufs=1, space="PSUM")

        pan = [pana, panb]  # per-sample accumulation bank

        # ---- warm-up: keep PE busy (and HAM un-throttled) during input DMA ----
        wsrc = consts.tile([P, LS], BF16, tag="wsrc", name="wsrc")
        nc.vector.memset(wsrc[:], 0.0)
        wps = macc.tile([P, LS], F32, tag="m", name="wps")
        for i in range(NWARM):
            nc.tensor.matmul(wps[:], wsrc[:, :P], wsrc[:], start=(i == 0), stop=(i == NWARM - 1))
        wsb = consts.tile([P, 4], F32, tag="wsb", name="wsb")
        nc.vector.tensor_copy(out=wsb[:], in_=wps[:, :4])
        nc.gpsimd.dma_start(warm_d[:], wsb[:])

        # ---- input x: spread DMA across the 3 DMA-capable queues, s0 first ----
        dmaq = [nc.sync, nc.scalar, nc.gpsimd]
        xp = {}
        for s in range(samples):
            for cb in range(CB):
                xp[s, cb] = xpp.tile([P, L], F32, tag=f"xp{s}{cb}", name=f"xp{s}{cb}")
        t2 = {}
        for s in range(samples):
            t2[s] = t2p.tile([P, CB, 2], F32, tag=f"t2{s}", name=f"t2{s}")
            nc.sync.dma_start(t2[s][:], t_d[s])
            qi = 0
            _QPAT = (1, 2, 0, 1, 2, 1, 2, 0)  # scalar/gpsimd HW queues take more
            for cb in range(CB):
                for i in range(2):
                    dmaq[_QPAT[qi % 8]].dma_start(
                        xp[s, cb][:, i * 2 * LS : (i + 1) * 2 * LS],
                        x_d[s, cb * P : (cb + 1) * P, i * 2 * LS : (i + 1) * 2 * LS],
                    )
                    qi += 1

        # ---- persistent constants / weights in SBUF (after x in queue order) ----
        w1_sb = {}
        w2_sb = {}
        for rb in ("r1", "r2"):
            w1_sb[rb] = consts.tile([P, CB, 3, C], BF16, tag=f"w1_{rb}", name=f"w1_{rb}")
            nc.scalar.dma_start(w1_sb[rb][:], w_conv[rb, 1][:])
            w2_sb[rb] = consts.tile([P, CB, 3, C], BF16, tag=f"w2_{rb}", name=f"w2_{rb}")
            nc.gpsimd.dma_start(w2_sb[rb][:], w_conv[rb, 2][:])
        wkqv_sb = consts.tile([P, CB, 3 * C], BF16, tag="wkqv", name="wkqv")
        nc.scalar.dma_start(wkqv_sb[:], wkqv_d[:])
        gnp_sb = {}
        for rb in ("r1", "r2"):
            for ln in (1, 2):
                for wb in ("w", "b"):
                    tl = consts.tile([P, CB], F32, tag=f"gn_{rb}{ln}{wb}", name=f"gn_{rb}{ln}{wb}")
                    nc.gpsimd.dma_start(tl[:], gnw_d[rb, ln, wb][:])
                    gnp_sb[rb, ln, wb] = tl
        c2b_sb = {}
        for rb, dten in c2b_d.items():
            c2b_sb[rb] = consts.tile([P, CB], F32, tag=f"c2b_{rb}", name=f"c2b_{rb}")
            nc.gpsimd.dma_start(c2b_sb[rb][:], dten[:])
        gind_sb = consts.tile([P, GPB], F32R, tag="gind", name="gind")
        nc.gpsimd.dma_start(gind_sb[:], gind_d[:])
        bind_sb = consts.tile([P, CB, P], F32R, tag="bind", name="bind")
        nc.gpsimd.dma_start(bind_sb[:], bind_d.rearrange("cb p c -> p cb c"))
        po_sb = consts.tile([P, 2, 16], F8, tag="po", name="po")
        nc.gpsimd.dma_start(po_sb[:], po_d[:])
        onesr_sb = consts.tile([1, P], F32R, tag="onesr", name="onesr")
        nc.gpsimd.dma_start(onesr_sb[:], onesr_d[:])
        eps_sb = consts.tile([P, 1], F32, tag="eps", name="eps")
        nc.vector.memset(eps_sb[:], EPS)
        expb_sb = consts.tile([P, 1], F32, tag="expb", name="expb")
        nc.vector.memset(expb_sb[:], EXPSHIFT)
        zero2 = consts.tile([P, 2], F32, tag="zero2", name="zero2")
        nc.vector.memset(zero2[:], 0.0)

        def alloc_act(tag):
            ts = []
            for cb in range(CB):
                tl = actp.tile([P, L + 2], BF16, tag=f"{tag}{cb}", name=f"{tag}{cb}")
                nc.vector.memset(tl[:, 0:1], 0.0)
                nc.vector.memset(tl[:, L + 1 : L + 2], 0.0)
                ts.append(tl)
            return ts

        def gn_relu(s, src, dst, rb, ln, dve_apply=False):
            """dst (padded bf16 act pair) = relu(groupnorm(src) * w + b)."""
            gp = []
            for cb in range(CB):
                stats = small.tile([P, NL, 6], F32, tag="stats", name="stats")
                for i in range(NL):
                    nc.vector.bn_stats(out=stats[:, i, :], in_=src[cb][:, i * LS : (i + 1) * LS])
                mv = small.tile([P, 2], F32, tag="mv", name="mv")
                nc.vector.bn_aggr(out=mv[:], in_=stats[:])
                tmp = small.tile([P, 2], F32R, tag="tmp", name="tmp")
                nc.gpsimd.tensor_copy(out=tmp[:, 0:1], in_=mv[:, 0:1])
                nc.gpsimd.tensor_tensor(out=tmp[:, 1:2], in0=mv[:, 0:1], in1=mv[:, 0:1], op=OP.mult)
                nc.gpsimd.tensor_tensor(out=tmp[:, 1:2], in0=tmp[:, 1:2].bitcast(F32), in1=mv[:, 1:2], op=OP.add)
                g = pan[s].tile([GPB, 2], F32, tag="a", name="gp")
                nc.tensor.matmul(g[:], gind_sb[:], tmp[:], start=True, stop=True)
                gp.append(g)
            NG = 32 * CB
            gs = small.tile([NG, 2], F32, tag="gs", name="gs")
            nc.vector.tensor_copy(out=gs[:], in_=zero2[:NG])
            for cb in range(CB):
                nc.vector.tensor_copy(out=gs[cb * 32 : cb * 32 + GPB, :], in_=gp[cb][:])
            var = small.tile([NG, 1], F32, tag="var", name="var")
            nc.vector.tensor_tensor(out=var[:], in0=gs[:, 0:1], in1=gs[:, 0:1], op=OP.mult)
            nc.vector.tensor_tensor(out=var[:], in0=gs[:, 1:2], in1=var[:], op=OP.subtract)
            nc.scalar.activation(out=var[:], in_=var[:], func=AF.Ln, bias=eps_sb[:NG])
            gpk = small.tile([P, 2], F32R, tag="gpk", name="gpk")
            nc.vector.tensor_copy(out=gpk[:], in_=zero2[:])
            nc.scalar.activation(out=gpk[:NG, 0:1], in_=var[:], func=AF.Exp, scale=-0.5)
            nc.vector.tensor_scalar_mul(gpk[:NG, 1:2], gs[:, 0:1], -1.0)
            for cb in range(CB):
                bc = pan[s].tile([P, 2], F32, tag="a", name="bc")
                nc.tensor.matmul(bc[:], bind_sb[:, cb, :], gpk[:], start=True, stop=True)
                sb = small.tile([P, 2], F32, tag="sb", name="sb")
                nc.vector.tensor_scalar_mul(sb[:, 0:1], bc[:, 0:1], gnp_sb[rb, ln, "w"][:, cb : cb + 1])
                nc.vector.tensor_tensor(out=sb[:, 1:2], in0=bc[:, 1:2], in1=sb[:, 0:1], op=OP.mult)
                nc.vector.tensor_scalar_add(sb[:, 1:2], sb[:, 1:2], gnp_sb[rb, ln, "b"][:, cb : cb + 1])
                for i in range(2):
                    dsl = dst[cb][:, 1 + i * 2 * LS : 1 + (i + 1) * 2 * LS]
                    ssl = src[cb][:, i * 2 * LS : (i + 1) * 2 * LS]
                    if dve_apply:
                        nc.vector.tensor_scalar(
                            dsl, ssl, sb[:, 0:1], sb[:, 1:2], op0=OP.mult, op1=OP.add
                        )
                        nc.vector.tensor_scalar_max(dsl, dsl, 0.0)
                    else:
                        nc.scalar.activation(
                            out=dsl, in_=ssl, func=AF.Relu,
                            bias=sb[:, 1:2], scale=sb[:, 0:1],
                        )

        def conv3(s, src, wt, consume):
            for ocb in range(CB):
                for ls in range(NL):
                    ps = macc.tile([P, LS], F32, tag="m", name="acc")
                    k = 0
                    for icb in range(CB):
                        for tap in range(3):
                            nc.tensor.matmul(
                                ps[:],
                                wt[:, icb, tap, ocb * P : (ocb + 1) * P],
                                src[icb][:, ls * LS + tap : ls * LS + tap + LS],
                                start=(k == 0),
                                stop=(k == 5),
                            )
                            k += 1
                    consume(ocb, ls, ps)

        # ---- res-block phase pieces (so emission can interleave samples) ----
        act1 = {}
        hbuf = {}
        act2 = {}
        x1 = {}

        def phase_gn1(s, rb, src):
            act1[s] = alloc_act("a")
            with nc.named_scope(f"s{s}_{rb}_gn1"):
                gn_relu(s, src, act1[s], rb, 1, dve_apply=(rb == "r2"))

        def phase_conv1(s, rb, rbi, eat1_act):
            hbuf[s] = [hp.tile([P, L], BF16, tag=f"h{cb}", name=f"h{cb}") for cb in range(CB)]
            with nc.named_scope(f"s{s}_{rb}_conv1"):
                def eat1(ocb, ls, ps):
                    dst = hbuf[s][ocb][:, ls * LS : (ls + 1) * LS]
                    if eat1_act:
                        nc.scalar.activation(out=dst, in_=ps[:], func=AF.Identity,
                                             bias=t2[s][:, ocb, rbi : rbi + 1])
                    else:
                        nc.vector.tensor_scalar_add(dst, ps[:], t2[s][:, ocb, rbi : rbi + 1])
                conv3(s, act1[s], w1_sb[rb], eat1)

        def phase_gn2(s, rb):
            act2[s] = alloc_act("a")
            with nc.named_scope(f"s{s}_{rb}_gn2"):
                gn_relu(s, hbuf[s], act2[s], rb, 2, dve_apply=(rb == "r2"))

        def phase_conv2(s, rb, src, final):
            res = None
            if not final:
                res = [x1p.tile([P, L], BF16, tag=f"x1{s}{cb}", name=f"x1{s}{cb}") for cb in range(CB)]
            with nc.named_scope(f"s{s}_{rb}_conv2"):
                oq = [0]
                def eat2(ocb, ls, ps):
                    if rb in c2b_sb:
                        nc.vector.tensor_scalar_add(ps[:], ps[:], c2b_sb[rb][:, ocb : ocb + 1])
                    resid = src[ocb][:, ls * LS : (ls + 1) * LS]
                    if final:
                        ot = outp.tile([P, LS], F32, tag="ot", name="ot")
                        nc.vector.tensor_tensor(out=ot[:], in0=ps[:], in1=resid, op=OP.add)
                        dmaq[oq[0] % 3].dma_start(
                            out_d[s, ocb * P : (ocb + 1) * P, ls * LS : (ls + 1) * LS], ot[:]
                        )
                        oq[0] += 1
                    else:
                        nc.vector.tensor_tensor(
                            out=res[ocb][:, ls * LS : (ls + 1) * LS],
                            in0=ps[:], in1=resid, op=OP.add,
                        )
                conv3(s, act2[s], w2_sb[rb], eat2)
            return res

        def kqv(s, x1s):
            """k,q into fp8 pair-plane layout; v into fp8 [l, kb, c] layout.
            2x[128,1024] psum pipe: 4 matmuls fill a tile, one DVE eviction."""
            ktp = kqvp.tile([P, CB, L], F8, tag=f"kt{s}", name=f"kt{s}")
            qtp = kqvp.tile([P, CB, L], F8, tag=f"qt{s}", name=f"qt{s}")
            vtp = kqvp.tile([P, KB, C], F8, tag=f"vt{s}", name=f"vt{s}")
            with nc.named_scope(f"s{s}_kqv"):
                for j, dst in ((0, ktp), (1, qtp)):
                    for ocb in range(CB):
                        off = j * C + ocb * P
                        for ls in range(NL):
                            ps = macc.tile([P, LS], F32, tag="m", name="ka")
                            for icb in range(CB):
                                nc.tensor.matmul(
                                    ps[:],
                                    wkqv_sb[:, icb, off : off + P],
                                    x1s[icb][:, ls * LS : (ls + 1) * LS],
                                    start=(icb == 0),
                                    stop=(icb == 1),
                                )
                            nc.vector.tensor_copy(
                                out=dst[:, ocb, ls * LS : (ls + 1) * LS], in_=ps[:]
                            )
                for lh in range(KB // 2):
                    ps = macc.tile([P, LS], F32, tag="m", name="ka")
                    for lsub in range(2):
                        lb = lh * 2 + lsub
                        for icb in range(CB):
                            nc.tensor.matmul(
                                ps[:, lsub * C : (lsub + 1) * C],
                                x1s[icb][:, lb * P : (lb + 1) * P],
                                wkqv_sb[:, icb, 2 * C : 3 * C],
                                start=(icb == 0),
                                stop=(icb == 1),
                            )
                    nc.vector.tensor_copy(out=vtp[:, lh * 2 : (lh + 1) * 2, :], in_=ps[:])
            return ktp, qtp, vtp

        def attn_qs(s, qs, ktp, qtp, vtp, av):
            with nc.named_scope(f"s{s}_attn{qs}"):
                ex = expp.tile([P, KB, LS], F8, tag="ex", name="ex")
                qsl = qtp[:, :, qs * LS : (qs + 1) * LS]
                # scores + exp, alternating 2-bank / 1-bank psum rounds
                for kb0, nkb in _ROUNDS:
                    sp = spsa.tile([P, nkb * LS], F32, tag="sp", name="sp")
                    for j in range(nkb):
                        kb = kb0 + j
                        nc.tensor.matmul(
                            sp[:, j * LS : (j + 1) * LS],
                            ktp[:, :, kb * P : (kb + 1) * P],
                            qsl,
                            start=True, stop=True, perf_mode=DR,
                        )
                    nc.scalar.activation(
                        out=ex[:, kb0 : kb0 + nkb, :], in_=sp[:],
                        func=AF.Exp, bias=expb_sb[:], scale=SCALE,
                    )
                # denominator (8 pair matmuls), then 1/dn broadcast
                dn = pan[s].tile([1, LS], F32, tag="a", name="dn")
                for g in range(KB // 2):
                    nc.tensor.matmul(
                        dn[:], po_sb[:, :, 0:1], ex[:, 2 * g : 2 * g + 2, :],
                        start=(g == 0), stop=(g == KB // 2 - 1), perf_mode=DR,
                    )
                lnd = small.tile([1, LS], F32, tag="lnd", name="lnd")
                nc.scalar.activation(out=lnd[:], in_=dn[:], func=AF.Ln)
                rd = small.tile([1, LS], F32R, tag="rd", name="rd")
                nc.scalar.activation(out=rd[:], in_=lnd[:], func=AF.Exp, scale=-1.0)
                rb_ps = pan[s].tile([P, LS], F32, tag="a", name="rb_ps")
                nc.tensor.matmul(rb_ps[:], onesr_sb[:], rd[:], start=True, stop=True)
                rdb = rdbp.tile([P, LS], BF16, tag="rdbs", name="rdb")
                nc.vector.tensor_copy(out=rdb[:], in_=rb_ps[:])
                # attn @ V per channel block (per-sample bank, sequential cb)
                for cb in range(CB):
                    psv = pan[s].tile([P, LS], F32, tag="a", name="psv")
                    for g in range(KB // 2):
                        nc.tensor.matmul(
                            psv[:],
                            vtp[:, 2 * g : 2 * g + 2, cb * P : (cb + 1) * P],
                            ex[:, 2 * g : 2 * g + 2, :],
                            start=(g == 0), stop=(g == KB // 2 - 1), perf_mode=DR,
                        )
                    nc.vector.tensor_tensor(
                        out=av[cb][:, qs * LS : (qs + 1) * LS],
                        in0=psv[:], in1=rdb[:], op=OP.mult,
                    )

        # ================= program body (samples interleaved) =================
        for s in range(samples):
            phase_gn1(s, "r1", [xp[s, 0], xp[s, 1]])
        for s in range(samples):
            phase_conv1(s, "r1", 0, eat1_act=False)
        for s in range(samples):
            phase_gn2(s, "r1")
        for s in range(samples):
            x1[s] = phase_conv2(s, "r1", [xp[s, 0], xp[s, 1]], final=False)
        kqvt = {}
        for s in range(samples):
            kqvt[s] = kqv(s, x1[s])
        av = {}
        for s in range(samples):
            av[s] = [avp.tile([P, L], BF16, tag=f"av{s}{cb}", name=f"av{s}{cb}") for cb in range(CB)]
        attn_qs(0, 0, *kqvt[0], av[0])
        attn_qs(0, 1, *kqvt[0], av[0])
        attn_qs(1, 0, *kqvt[1], av[1])
        attn_qs(0, 2, *kqvt[0], av[0])
        attn_qs(1, 1, *kqvt[1], av[1])
        attn_qs(0, 3, *kqvt[0], av[0])
        attn_qs(1, 2, *kqvt[1], av[1])
        phase_gn1(0, "r2", av[0])
        attn_qs(1, 3, *kqvt[1], av[1])
        phase_conv1(0, "r2", 1, eat1_act=True)
        phase_gn1(1, "r2", av[1])
        phase_gn2(0, "r2")
        phase_conv2(0, "r2", av[0], final=True)
        phase_conv1(1, "r2", 1, eat1_act=True)
        phase_gn2(1, "r2")
        phase_conv2(1, "r2", av[1], final=True)

    nc.finalize()
    return nc


def _to_bf16(a):
    a = np.ascontiguousarray(np.asarray(a, np.float32))
    u = a.view(np.uint32)
    return (((u >> 16) + ((u >> 15) & 1)) & 0xFFFF).astype(np.uint16)


def _pack_conv_w(w):
    """(O, I, 3) f32 -> [P, icb, tap, oc] bf16 (uint16 view)."""
    w = np.asarray(w, dtype=np.float32)
    t = np.ascontiguousarray(w.transpose(1, 2, 0).reshape(CB, P, 3, w.shape[0]).transpose(1, 0, 2, 3))
    return _to_bf16(t)


def _pack_gn(v):
    return np.ascontiguousarray(np.asarray(v, dtype=np.float32).reshape(CB, P).T)


def make_in_maps(inp, use_bias):
    gind = np.zeros((P, GPB), np.float32)
    bind = np.zeros((CB, P, P), np.float32)
    for cc in range(P):
        gind[cc, cc // 8] = 0.125
        for cb in range(CB):
            bind[cb, cb * 32 + cc // 8, cc] = 1.0
    po = np.full((P, 2, 16), 0x38, np.uint8)  # fp8e4 1.0
    shared = {
        "wkqvt": _to_bf16(
            inp["lin_w"][:, :, 0].T.reshape(CB, P, 3 * C).transpose(1, 0, 2)
        ),
        "gind": gind,
        "bind": bind,
        "po": po,
        "onesr": np.ones((1, P), np.float32),
    }
    for rb in ("r1", "r2"):
        shared[f"{rb}_w1t"] = _pack_conv_w(inp[f"{rb}_c1_w"])
        shared[f"{rb}_w2t"] = _pack_conv_w(inp[f"{rb}_c2_w"])
        for ln in (1, 2):
            shared[f"{rb}_gn{ln}_ws"] = _pack_gn(inp[f"{rb}_gn{ln}_w"])
            shared[f"{rb}_gn{ln}_bs"] = _pack_gn(inp[f"{rb}_gn{ln}_b"])
    if "c2b_r1" in use_bias:
        shared["r1_c2bs"] = _pack_gn(inp["r1_c2_b"])
    if "c2b_r2" in use_bias:
        shared["r2_c2bs"] = _pack_gn(inp["r2_c2_b"])

    tfull = inp["t"][:, :, 0]
    nb = inp["x"].shape[0]
    tv = np.empty((nb, P, CB, 2), np.float32)
    for rbi, rb in enumerate(("r1", "r2")):
        v = tfull + inp[f"{rb}_c1_b"][None, :]
        tv[:, :, :, rbi] = v.reshape(nb, CB, P).transpose(0, 2, 1)

    in_maps = []
    for c in range(NCORES):
        sl = slice(S * c, S * (c + 1))
        m = dict(shared)
        m["x"] = inp["x"][sl]
        m["tv"] = np.ascontiguousarray(tv[sl])
        in_maps.append(m)
    return in_maps


_CACHE = {}


def kernel(**inputs):
    inp = {k: np.ascontiguousarray(np.asarray(v, dtype=np.float32)) for k, v in inputs.items()}

    use_bias = []
    if np.any(inp["r1_c2_b"]):
        use_bias.append("c2b_r1")
    if np.any(inp["r2_c2_b"]):
        use_bias.append("c2b_r2")
    if np.any(inp["lin_b"]):
        use_bias.append("linb")
    use_bias = tuple(use_bias)

    if ("nc", use_bias) not in _CACHE:
        _CACHE[("nc", use_bias)] = build_program(S, use_bias)
    nc = _CACHE[("nc", use_bias)]

    in_maps = make_in_maps(inp, use_bias)
    res = _bu.run_bass_kernel_spmd(nc, in_maps, core_ids=list(range(NCORES)))
    out = np.concatenate([res.results[c]["out"] for c in range(NCORES)], axis=0)
    return out.astype(np.float32)
